# revision 1
# baseline (speedup 1.0000x reference)
"""Self-contained Trainium2 Bass kernel for nn_GAT_Linear (2x GATConv + mean-pool + MLP).

Strategy (8 NeuronCores, graph/data parallel by destination node):
  * Nodes are sharded contiguously at graph boundaries (batch is sorted), so
    each core owns ~8 graphs / ~6250 nodes and every edge whose dst lands there.
  * Per layer, each core computes the feature table for its own node slice with
    one matmul against host-fused weights ([W@att_dst | W@att_src | W]), then
    AllGathers the bf16 table so all cores can gather arbitrary source rows.
  * Message passing runs per 128-dst-node tile: batched dma_gather pulls the
    per-edge source rows (and dst attention coefficients), the segment softmax
    becomes exp(leaky_relu(logits)) with the normalization deferred past the
    aggregation, and the scatter-add is a one-hot matmul on the tensor engine
    (S^T[edge, dst_local] built on-device from host-provided local dst ids).
    Numerator and denominator accumulate in one PSUM matmul per edge subtile.
  * Mean-pool is one more small matmul per tile against a host {0,1} pool
    matrix; the tiny MLP head runs on host (negligible FLOPs).
  * dma_gather needs int16 indices, so each table is split into an A part
    (<=32768 rows) and a B part, with edges routed to the matching gather.
"""
import numpy as np
import ml_dtypes

import concourse.bass as bass
import concourse.mybir as mybir
import concourse.tile as tile
from concourse import library_config
from concourse.library_overlay import lower_extended_insts
from concourse.bass_utils import run_bass_kernel_spmd

# ---- problem constants (hardcoded per task contract) ----
N = 50000
E = 800000
IN_DIM = 256
HID = 64
HEADS = 4
H2 = 32
OUT_DIM = 16
G = 64
CORES = 8
GPC = G // CORES
NEG = 0.2
P = 128

ROW1 = 384   # L1 table row (bf16): [a_src(4) | a_dst(4) | h(256) | pad] = 768B
ROW2 = 128   # L2 table row (bf16): [a_src2(1) | a_dst2(1) | h2(32) | pad] = 256B
NTPA = 4096  # per-core rows in table A; preprocess rebalances to ~NTP/2

bf16 = mybir.dt.bfloat16
fp32 = mybir.dt.float32
i16 = mybir.dt.int16
AF = mybir.ActivationFunctionType
OP = mybir.AluOpType


def _split_waits(nc, limit=1):
    """This walrus build rejects >1 sync-wait per instruction; hoist extras
    onto preceding same-engine NOPs (engine streams are in-order)."""
    n = 0
    for fn in nc.m.functions:
        for blk in fn.blocks:
            new_insts = []
            for inst in blk.instructions:
                si = getattr(inst, "sync_info", None)
                waits = list(si.on_wait) if si is not None else []
                if len(waits) > limit:
                    hoist, keep = waits[:-limit], waits[-limit:]
                    for j, w in enumerate(hoist):
                        nop = mybir.InstNoOp(
                            name=f"{inst.name}_wsplit{j}", ins=[], outs=[],
                            text_hint="waitsplit")
                        nop.engine = inst.engine
                        nop.sync_info = mybir.SyncInfo(on_wait=[w], on_update=[])
                        new_insts.append(nop)
                        n += 1
                    si.on_wait = keep
                new_insts.append(inst)
            blk.instructions[:] = new_insts
    return n


def build_nc(NT, KA, KB, emit_b1, emit_b2, reps=1, stop_after="Z"):
    """Build the SPMD kernel (same program for all 8 cores)."""
    NTP = NT * P
    NTPB = NTP - NTPA
    K = KA + KB
    NIA, NIB, NI = KA * P, KB * P, K * P

    nc = bass.Bass()

    # ---- I/O ----
    xt_d = nc.dram_tensor("xt", [IN_DIM, NTP], bf16, kind="ExternalInput")
    w1e_d = nc.dram_tensor("w1e", [IN_DIM, 264], bf16, kind="ExternalInput")
    w2e_d = nc.dram_tensor("w2e", [IN_DIM, 34], bf16, kind="ExternalInput")
    idxa_d = nc.dram_tensor("idxa", [NT, P, max(NIA // 16, 1)], i16, kind="ExternalInput")
    idxb_d = nc.dram_tensor("idxb", [NT, P, max(NIB // 16, 1)], i16, kind="ExternalInput")
    loc_d = nc.dram_tensor("loc", [NT, P, K], bf16, kind="ExternalInput")
    sexp_d = nc.dram_tensor("sexp", [NT, P, K, P], bf16, kind="ExternalInput")
    pool_d = nc.dram_tensor("poolm", [NT, P, GPC], bf16, kind="ExternalInput")
    b1_d = nc.dram_tensor("b1r", [P, HEADS * HID], bf16, kind="ExternalInput")
    b2_d = nc.dram_tensor("b2r", [P, H2], bf16, kind="ExternalInput")
    iota_d = nc.dram_tensor("iotar", [P, P], bf16, kind="ExternalInput")
    ident_d = nc.dram_tensor("identr", [P, P], bf16, kind="ExternalInput")
    out_d = nc.dram_tensor("pooled", [GPC, H2], fp32, kind="ExternalOutput")

    # ---- internal DRAM (collectives) ----
    ag1a_in = nc.dram_tensor("ag1a_in", [NTPA, ROW1], bf16)
    ag1b_in = nc.dram_tensor("ag1b_in", [NTPB, ROW1], bf16)
    tabA = nc.dram_tensor("tabA", [CORES * NTPA, ROW1], bf16, addr_space="Shared")
    tabB = nc.dram_tensor("tabB", [CORES * NTPB, ROW1], bf16, addr_space="Shared")
    ag2a_in = nc.dram_tensor("ag2a_in", [NTPA, ROW2], bf16)
    ag2b_in = nc.dram_tensor("ag2b_in", [NTPB, ROW2], bf16)
    tab2A = nc.dram_tensor("tab2A", [CORES * NTPA, ROW2], bf16, addr_space="Shared")
    tab2B = nc.dram_tensor("tab2B", [CORES * NTPB, ROW2], bf16, addr_space="Shared")

    rg = [list(range(CORES))]

    with tile.TileContext(nc) as tc:
        with (
            tc.tile_pool(name="const", bufs=1) as cst,
            tc.tile_pool(name="io", bufs=3) as io,
            tc.tile_pool(name="wk", bufs=2) as wk,
            tc.tile_pool(name="gth", bufs=3) as gth,
            tc.tile_pool(name="big", bufs=1) as big,
            tc.tile_pool(name="ps", bufs=2, space="PSUM") as ps,
            tc.tile_pool(name="psg", bufs=1, space="PSUM") as psg,
        ):
            nc.gpsimd.load_library(library_config.mlp)
            nia_reg = nc.gpsimd.to_reg(NIA) if NIA else None
            nib_reg = nc.gpsimd.to_reg(NIB) if NIB else None

            iota_b = cst.tile([P, P], bf16)
            nc.sync.dma_start(out=iota_b[:], in_=iota_d[:])
            ident = cst.tile([P, P], bf16)
            nc.sync.dma_start(out=ident[:], in_=ident_d[:])

            w1e_sb = cst.tile([P, 2, 264], bf16)
            nc.sync.dma_start(out=w1e_sb[:], in_=w1e_d[:].rearrange("(a p) c -> p a c", p=P))
            w2e_sb = cst.tile([P, 2, 34], bf16)
            nc.sync.dma_start(out=w2e_sb[:], in_=w2e_d[:].rearrange("(a p) c -> p a c", p=P))
            xt_sb = big.tile([P, 2, NTP], bf16)
            nc.sync.dma_start(out=xt_sb[:], in_=xt_d[:].rearrange("(a p) c -> p a c", p=P))
            if emit_b1:
                b1_sb = cst.tile([P, HEADS * HID], bf16)
                nc.sync.dma_start(out=b1_sb[:], in_=b1_d[:])
            if emit_b2:
                b2_sb = cst.tile([P, H2], bf16)
                nc.sync.dma_start(out=b2_sb[:], in_=b2_d[:])

            for rep in range(reps):
                # ================= Phase A: table1 slice =================
                for t in range(NT):
                    acc = ps.tile([P, 264], fp32, tag="acc")
                    for kk in range(2):
                        nc.tensor.matmul(
                            acc[:], xt_sb[:, kk, t * P:(t + 1) * P], w1e_sb[:, kk, :],
                            start=(kk == 0), stop=(kk == 1))
                    rowt = wk.tile([P, 264], bf16, tag="rowt")
                    nc.vector.tensor_copy(rowt[:], acc[:])
                    if t < NTPA // P:
                        nc.sync.dma_start(
                            out=ag1a_in[t * P:(t + 1) * P, 0:264], in_=rowt[:])
                    else:
                        o = t * P - NTPA
                        nc.sync.dma_start(
                            out=ag1b_in[o:o + P, 0:264], in_=rowt[:])
                nc.gpsimd.collective_compute(
                    "AllGather", OP.bypass, ins=[ag1a_in[:]], outs=[tabA[:]],
                    replica_groups=rg)
                nc.gpsimd.collective_compute(
                    "AllGather", OP.bypass, ins=[ag1b_in[:]], outs=[tabB[:]],
                    replica_groups=rg)
                if stop_after == "A":
                    break

                # ============ Phase B: L1 message passing ============
                out1T = big.tile([P, 2, NTP], bf16)
                for t in range(NT):
                    g_all = gth.tile([P, K, ROW1], bf16, tag="g1")
                    if NIA:
                        idx_a = io.tile([P, NIA // 16], i16, tag="ixa")
                        nc.sync.dma_start(out=idx_a[:], in_=idxa_d[t])
                        nc.gpsimd.dma_gather(
                            out_ap=g_all[:, 0:KA, :], in_ap=tabA[:], idxs_ap=idx_a[:],
                            num_idxs=NIA, num_idxs_reg=nia_reg, elem_size=ROW1,
                            single_packet=False)
                    if NIB:
                        idx_b = io.tile([P, NIB // 16], i16, tag="ixb")
                        nc.sync.dma_start(out=idx_b[:], in_=idxb_d[t])
                        nc.gpsimd.dma_gather(
                            out_ap=g_all[:, KA:K, :], in_ap=tabB[:], idxs_ap=idx_b[:],
                            num_idxs=NIB, num_idxs_reg=nib_reg, elem_size=ROW1,
                            single_packet=False)
                    # a_dst expansion: host one-hot S[node, edge] @ local a_dst
                    sexp_t = wk.tile([P, K, P], bf16, tag="sx")
                    nc.sync.dma_start(out=sexp_t[:], in_=sexp_d[t])
                    adc = io.tile([P, HEADS], bf16, tag="adc")
                    if t < NTPA // P:
                        nc.sync.dma_start(
                            out=adc[:], in_=ag1a_in[t * P:(t + 1) * P, 4:8])
                    else:
                        o = t * P - NTPA
                        nc.sync.dma_start(out=adc[:], in_=ag1b_in[o:o + P, 4:8])
                    adx = ps.tile([P, HEADS * K], fp32, tag="adx")
                    for j in range(K):
                        nc.tensor.matmul(
                            adx[:, 4 * j:4 * (j + 1)], sexp_t[:, j, :], adc[:],
                            start=True, stop=True)

                    loc_t = io.tile([P, K], bf16, tag="loc")
                    nc.sync.dma_start(out=loc_t[:], in_=loc_d[t])
                    s_all = wk.tile([P, K, P], bf16, tag="sel")
                    nc.vector.tensor_tensor(
                        out=s_all[:],
                        in0=loc_t[:].unsqueeze(2).to_broadcast([P, K, P]),
                        in1=iota_b[:].unsqueeze(1).to_broadcast([P, K, P]),
                        op=OP.is_equal)

                    logit = wk.tile([P, K, HEADS], fp32, tag="lg1")
                    nc.vector.tensor_tensor(
                        out=logit[:], in0=g_all[:, :, 0:4],
                        in1=adx[:].rearrange("p (k h) -> p k h", h=HEADS),
                        op=OP.add)
                    lsc = wk.tile([P, K, HEADS], fp32, tag="ls1")
                    nc.vector.tensor_scalar(
                        out=lsc[:], in0=logit[:], scalar1=NEG, scalar2=None, op0=OP.mult)
                    lrl = wk.tile([P, K, HEADS], fp32, tag="lr1")
                    nc.vector.tensor_tensor(out=lrl[:], in0=logit[:], in1=lsc[:], op=OP.max)
                    exp_a = wk.tile([P, K, HEADS], bf16, tag="ex1")
                    nc.scalar.activation(out=exp_a[:], in_=lrl[:], func=AF.Exp)

                    v_all = wk.tile([P, K, 260], bf16, tag="v1")
                    nc.vector.tensor_copy(v_all[:, :, 0:4], exp_a[:])
                    for h in range(HEADS):
                        nc.vector.tensor_tensor(
                            out=v_all[:, :, 4 + 64 * h:4 + 64 * (h + 1)],
                            in0=g_all[:, :, 8 + 64 * h:8 + 64 * (h + 1)],
                            in1=exp_a[:, :, h:h + 1].to_broadcast([P, K, 64]),
                            op=OP.mult)

                    acc = ps.tile([P, 264], fp32, tag="acc")
                    for j in range(K):
                        nc.tensor.matmul(
                            acc[:, 0:260], s_all[:, j, :], v_all[:, j, :],
                            start=(j == 0), stop=(j == K - 1))

                    dnm = wk.tile([P, 4], fp32, tag="dn1")
                    nc.vector.tensor_scalar(
                        out=dnm[:], in0=acc[:, 0:4], scalar1=1e-16, scalar2=None,
                        op0=OP.add)
                    rec = wk.tile([P, 4], fp32, tag="rc1")
                    nc.vector.reciprocal(rec[:], dnm[:])
                    h1r = wk.tile([P, 256], bf16, tag="h1r")
                    for h in range(HEADS):
                        nc.vector.tensor_tensor(
                            out=h1r[:, 64 * h:64 * (h + 1)],
                            in0=acc[:, 4 + 64 * h:4 + 64 * (h + 1)],
                            in1=rec[:, h:h + 1].to_broadcast([P, 64]),
                            op=OP.mult)
                    if emit_b1:
                        nc.vector.tensor_tensor(
                            out=h1r[:], in0=h1r[:], in1=b1_sb[:], op=OP.add)
                    nc.vector.tensor_scalar(
                        out=h1r[:], in0=h1r[:], scalar1=0.0, scalar2=None, op0=OP.max)

                    for kk in range(2):
                        tr = ps.tile([P, P], bf16, tag="tr")
                        nc.tensor.transpose(
                            out=tr[:], in_=h1r[:, kk * P:(kk + 1) * P],
                            identity=ident[:])
                        nc.vector.tensor_copy(out1T[:, kk, t * P:(t + 1) * P], tr[:])

                if stop_after == "B":
                    break
                # ============ Phase C: table2 slice ============
                # (kept as a separate loop: fusing it into Phase B measured
                # WORSE — it perturbs the gather-bound pipeline's schedule)
                for t in range(NT):
                    acc2 = ps.tile([P, 34], fp32, tag="acc")
                    for kk in range(2):
                        nc.tensor.matmul(
                            acc2[:], out1T[:, kk, t * P:(t + 1) * P], w2e_sb[:, kk, :],
                            start=(kk == 0), stop=(kk == 1))
                    rw2 = wk.tile([P, 34], bf16, tag="rw2")
                    nc.vector.tensor_copy(rw2[:], acc2[:])
                    if t < NTPA // P:
                        nc.sync.dma_start(
                            out=ag2a_in[t * P:(t + 1) * P, 0:34], in_=rw2[:])
                    else:
                        o = t * P - NTPA
                        nc.sync.dma_start(out=ag2b_in[o:o + P, 0:34], in_=rw2[:])
                nc.gpsimd.collective_compute(
                    "AllGather", OP.bypass, ins=[ag2a_in[:]], outs=[tab2A[:]],
                    replica_groups=rg)
                nc.gpsimd.collective_compute(
                    "AllGather", OP.bypass, ins=[ag2b_in[:]], outs=[tab2B[:]],
                    replica_groups=rg)

                if stop_after == "C":
                    break
                # ======== Phase D: L2 message passing + pooling ========
                gacc = psg.tile([GPC, H2], fp32, tag="gacc")
                for t in range(NT):
                    g2 = gth.tile([P, K, ROW2], bf16, tag="g2")
                    if NIA:
                        idx_a = io.tile([P, NIA // 16], i16, tag="ixa")
                        nc.sync.dma_start(out=idx_a[:], in_=idxa_d[t])
                        nc.gpsimd.dma_gather(
                            out_ap=g2[:, 0:KA, :], in_ap=tab2A[:], idxs_ap=idx_a[:],
                            num_idxs=NIA, num_idxs_reg=nia_reg, elem_size=ROW2,
                            single_packet=False)
                    if NIB:
                        idx_b = io.tile([P, NIB // 16], i16, tag="ixb")
                        nc.sync.dma_start(out=idx_b[:], in_=idxb_d[t])
                        nc.gpsimd.dma_gather(
                            out_ap=g2[:, KA:K, :], in_ap=tab2B[:], idxs_ap=idx_b[:],
                            num_idxs=NIB, num_idxs_reg=nib_reg, elem_size=ROW2,
                            single_packet=False)
                    sexp_t = wk.tile([P, K, P], bf16, tag="sx")
                    nc.sync.dma_start(out=sexp_t[:], in_=sexp_d[t])
                    adc2 = io.tile([P, 1], bf16, tag="adc2")
                    if t < NTPA // P:
                        nc.sync.dma_start(
                            out=adc2[:], in_=ag2a_in[t * P:(t + 1) * P, 1:2])
                    else:
                        o = t * P - NTPA
                        nc.sync.dma_start(out=adc2[:], in_=ag2b_in[o:o + P, 1:2])
                    adx2 = ps.tile([P, K], fp32, tag="adx")
                    for j in range(K):
                        nc.tensor.matmul(
                            adx2[:, j:j + 1], sexp_t[:, j, :], adc2[:],
                            start=True, stop=True)

                    loc_t = io.tile([P, K], bf16, tag="loc")
                    nc.sync.dma_start(out=loc_t[:], in_=loc_d[t])
                    s_all = wk.tile([P, K, P], bf16, tag="sel")
                    nc.vector.tensor_tensor(
                        out=s_all[:],
                        in0=loc_t[:].unsqueeze(2).to_broadcast([P, K, P]),
                        in1=iota_b[:].unsqueeze(1).to_broadcast([P, K, P]),
                        op=OP.is_equal)

                    lg2 = wk.tile([P, K, 1], fp32, tag="lg2")
                    nc.vector.tensor_tensor(
                        out=lg2[:], in0=g2[:, :, 0:1],
                        in1=adx2[:].unsqueeze(2), op=OP.add)
                    ls2 = wk.tile([P, K, 1], fp32, tag="ls2")
                    nc.vector.tensor_scalar(
                        out=ls2[:], in0=lg2[:], scalar1=NEG, scalar2=None, op0=OP.mult)
                    lr2 = wk.tile([P, K, 1], fp32, tag="lr2")
                    nc.vector.tensor_tensor(out=lr2[:], in0=lg2[:], in1=ls2[:], op=OP.max)
                    ex2 = wk.tile([P, K, 1], bf16, tag="ex2")
                    nc.scalar.activation(out=ex2[:], in_=lr2[:], func=AF.Exp)

                    v2 = wk.tile([P, K, 33], bf16, tag="v2")
                    nc.vector.tensor_copy(v2[:, :, 0:1], ex2[:])
                    nc.vector.tensor_tensor(
                        out=v2[:, :, 1:33], in0=g2[:, :, 2:34],
                        in1=ex2[:].to_broadcast([P, K, 32]), op=OP.mult)

                    acc2 = ps.tile([P, 34], fp32, tag="acc")
                    for j in range(K):
                        nc.tensor.matmul(
                            acc2[:, 0:33], s_all[:, j, :], v2[:, j, :],
                            start=(j == 0), stop=(j == K - 1))

                    dn2 = wk.tile([P, 1], fp32, tag="dn2")
                    nc.vector.tensor_scalar(
                        out=dn2[:], in0=acc2[:, 0:1], scalar1=1e-16, scalar2=None,
                        op0=OP.add)
                    rc2 = wk.tile([P, 1], fp32, tag="rc2")
                    nc.vector.reciprocal(rc2[:], dn2[:])
                    h2p = wk.tile([P, H2], bf16, tag="h2p")
                    nc.vector.tensor_tensor(
                        out=h2p[:], in0=acc2[:, 1:33],
                        in1=rc2[:].to_broadcast([P, H2]), op=OP.mult)
                    if emit_b2:
                        nc.vector.tensor_tensor(
                            out=h2p[:], in0=h2p[:], in1=b2_sb[:], op=OP.add)

                    pm = io.tile([P, GPC], bf16, tag="pm")
                    nc.sync.dma_start(out=pm[:], in_=pool_d[t])
                    nc.tensor.matmul(
                        gacc[:], pm[:], h2p[:], start=(t == 0), stop=(t == NT - 1))

                pooled = wk.tile([GPC, H2], fp32, tag="pooled")
                nc.vector.tensor_copy(pooled[:], gacc[:])
                nc.sync.dma_start(out=out_d[:], in_=pooled[:])

    lower_extended_insts(nc)
    _split_waits(nc)
    return nc


def _wrap16(idx_flat):
    """[NI] -> [P, NI//16] int16: position i -> partition i%16, col i//16,
    replicated across the 8 GPSIMD core groups."""
    ni = idx_flat.shape[0]
    w = np.zeros((P, max(ni // 16, 1)), np.int16)
    if ni:
        base = idx_flat.reshape(-1, 16).T.astype(np.int16)  # [16, ni/16]
        for grp in range(8):
            w[grp * 16:(grp + 1) * 16, :] = base
    return w


def preprocess(x, edge_index, batch, W1, att_src1, att_dst1, W2, att_src2, att_dst2):
    global NTPA
    n0 = np.searchsorted(batch, np.arange(0, G + 1, GPC)).astype(np.int64)
    nodes_c = n0[1:] - n0[:-1]
    NT = int(np.ceil(nodes_c.max() / P))
    NTP = NT * P
    # split tables near 50/50 so the per-half subtile padding (KA/KB) balances;
    # both halves must stay within int16 row-id range (8 * rows <= 32768)
    NTPA = min((NTP // 256) * P, 4096)
    NTPA = max(NTPA, NTP - 4096 + P) if NTP - NTPA > 4096 else NTPA
    NTPA = max(P, NTPA)
    NTPB = NTP - NTPA

    src = np.concatenate([edge_index[0], np.arange(N, dtype=np.int64)])
    dst = np.concatenate([edge_index[1], np.arange(N, dtype=np.int64)])

    owner = np.searchsorted(n0, np.arange(N), side="right") - 1
    ld_of = np.arange(N) - n0[owner]
    # A/B row ids in the AllGather'd tables
    isA = ld_of < NTPA
    rowid = np.where(isA, owner * NTPA + ld_of, owner * NTPB + (ld_of - NTPA))

    percore = []
    KA = KB = 0
    for c in range(CORES):
        lo, hi = n0[c], n0[c + 1]
        sel = (dst >= lo) & (dst < hi)
        es, ed = src[sel], dst[sel]
        order = np.argsort(ed, kind="stable")
        es, ed = es[order], ed[order]
        ld = ed - lo
        tid = ld >> 7
        ea = isA[es]
        cntA = np.bincount(tid[ea], minlength=NT)
        cntB = np.bincount(tid[~ea], minlength=NT)
        KA = max(KA, int(np.ceil(cntA.max() / P)))
        KB = max(KB, int(np.ceil(cntB.max() / P)))
        percore.append((es, ld, tid, ea))

    K = KA + KB
    NIA, NIB, NI = KA * P, KB * P, K * P
    data = []
    for c in range(CORES):
        es, ld, tid, ea = percore[c]
        idxa = np.zeros((NT, P, max(NIA // 16, 1)), np.int16)
        idxb = np.zeros((NT, P, max(NIB // 16, 1)), np.int16)
        loc = np.full((NT, K * P), 200.0, np.float32)
        for t in range(NT):
            m = tid == t
            iA = np.zeros(NIA, np.int64)
            iB = np.zeros(NIB, np.int64)
            locf = np.full(NI, 200.0, np.float32)
            mA = m & ea
            mB = m & ~ea
            nA, nB = int(mA.sum()), int(mB.sum())
            iA[:nA] = rowid[es[mA]]
            iB[:nB] = rowid[es[mB]]
            # slot order: A edges occupy subtiles [0,KA), B edges [KA,K)
            locf[:nA] = ld[mA] & 127
            locf[NIA:NIA + nB] = ld[mB] & 127
            idxa[t] = _wrap16(iA)
            idxb[t] = _wrap16(iB)
            loc[t] = locf
        # pool matrix
        lo, hi = n0[c], n0[c + 1]
        pm = np.zeros((NTP, GPC), np.float32)
        gl = batch[lo:hi] - c * GPC
        pm[np.arange(hi - lo), gl] = 1.0
        xt = np.zeros((IN_DIM, NTP), np.float32)
        xt[:, :hi - lo] = x[lo:hi].T
        lockp = loc.reshape(NT, K, P)
        sexp = (np.arange(P)[None, :, None, None] == lockp[:, None, :, :])
        data.append(dict(
            idxa=idxa, idxb=idxb,
            loc=lockp.transpose(0, 2, 1).astype(ml_dtypes.bfloat16),
            sexp=sexp.astype(ml_dtypes.bfloat16),
            poolm=pm.reshape(NT, P, GPC).astype(ml_dtypes.bfloat16),
            xt=xt.astype(ml_dtypes.bfloat16),
        ))
    cnts = np.stack([
        np.bincount(batch[n0[c]:n0[c + 1]] - c * GPC, minlength=GPC)
        for c in range(CORES)]).astype(np.float64)

    # fused weights: [W@att_dst | W@att_src | W]
    A1s = np.stack([W1[:, h * HID:(h + 1) * HID] @ att_src1[h] for h in range(HEADS)], 1)
    A1d = np.stack([W1[:, h * HID:(h + 1) * HID] @ att_dst1[h] for h in range(HEADS)], 1)
    W1e = np.concatenate([A1s, A1d, W1], axis=1).astype(ml_dtypes.bfloat16)
    A2s = (W2 @ att_src2[0])[:, None]
    A2d = (W2 @ att_dst2[0])[:, None]
    W2e = np.concatenate([A2s, A2d, W2], axis=1).astype(ml_dtypes.bfloat16)

    return dict(n0=n0, NT=NT, KA=KA, KB=KB, data=data, cnts=cnts, W1e=W1e, W2e=W2e)


def make_in_maps(pp, b1, b2):
    b1r = np.tile(b1[None, :], (P, 1)).astype(ml_dtypes.bfloat16)
    b2r = np.tile(b2[None, :], (P, 1)).astype(ml_dtypes.bfloat16)
    in_maps = []
    for c in range(CORES):
        d = pp["data"][c]
        in_maps.append({
            "xt": d["xt"], "w1e": pp["W1e"], "w2e": pp["W2e"],
            "idxa": d["idxa"], "idxb": d["idxb"], "sexp": d["sexp"],
            "loc": d["loc"], "poolm": d["poolm"], "b1r": b1r, "b2r": b2r,
            "iotar": np.tile(np.arange(P, dtype=np.float32), (P, 1)).astype(ml_dtypes.bfloat16),
            "identr": np.eye(P, dtype=np.float32).astype(ml_dtypes.bfloat16),
        })
    return in_maps


def epilogue(pooled_sums, pp, fc1_w, fc1_b, fc2_w, fc2_b):
    """pooled_sums: [CORES, GPC, H2] -> final [G, OUT_DIM] fp32."""
    cnts = pp["cnts"]  # [CORES, GPC]
    g = pooled_sums.astype(np.float64) / np.maximum(cnts, 1.0)[:, :, None]
    g = g.reshape(G, H2)
    g = np.maximum(g, 0.0)
    g = np.maximum(g @ fc1_w.astype(np.float64) + fc1_b, 0.0)
    g = np.maximum(g @ fc2_w.astype(np.float64) + fc2_b, 0.0)
    return g.astype(np.float32)


def kernel(**inputs):
    x = np.asarray(inputs["x"], np.float32)
    edge_index = np.asarray(inputs["edge_index"], np.int64)
    batch = np.asarray(inputs["batch"], np.int64)
    W1 = np.asarray(inputs["W1"], np.float32)
    att_src1 = np.asarray(inputs["att_src1"], np.float32)
    att_dst1 = np.asarray(inputs["att_dst1"], np.float32)
    b1 = np.asarray(inputs["b1"], np.float32)
    W2 = np.asarray(inputs["W2"], np.float32)
    att_src2 = np.asarray(inputs["att_src2"], np.float32)
    att_dst2 = np.asarray(inputs["att_dst2"], np.float32)
    b2 = np.asarray(inputs["b2"], np.float32)

    pp = preprocess(x, edge_index, batch, W1, att_src1, att_dst1, W2, att_src2, att_dst2)
    emit_b1 = bool(np.any(b1))
    emit_b2 = bool(np.any(b2))
    nc = build_nc(pp["NT"], pp["KA"], pp["KB"], emit_b1, emit_b2)
    in_maps = make_in_maps(pp, b1, b2)
    res = run_bass_kernel_spmd(nc, in_maps, list(range(CORES))).results
    pooled = np.stack([res[c]["pooled"] for c in range(CORES)])
    return epilogue(pooled, pp,
                    np.asarray(inputs["fc1_w"], np.float32), np.asarray(inputs["fc1_b"], np.float32),
                    np.asarray(inputs["fc2_w"], np.float32), np.asarray(inputs["fc2_b"], np.float32))



# revision 3
# speedup vs baseline: 1.4910x; 1.4910x over previous
"""Self-contained Trainium2 Bass kernel for nn_GAT_Linear (2x GATConv + mean-pool + MLP).

Strategy (8 NeuronCores, graph/data parallel by destination node):
  * Nodes sharded contiguously at graph boundaries (batch sorted); each core
    owns ~8 graphs / ~6250 nodes and every non-self edge whose dst lands there.
  * Per layer, each core computes its feature-table slice with one matmul
    against host-fused weights ([W@att_src | W@att_dst | W]), then ONE merged
    AllGather of the bf16 table so all cores can gather arbitrary source rows.
  * Message passing per 128-dst-node tile: batched dma_gather pulls per-edge
    source rows, segment softmax becomes exp(leaky_relu(logits)) with the
    normalization deferred past aggregation, and the scatter-add is a one-hot
    matmul on the tensor engine. Appended self-loops never enter the edge
    stream: their contribution is added analytically from the local table.
  * Layer-1 h uses an interleaved (channel-major, head-minor) layout so the
    exp-scale multiply and division run as single DVE ops in 2x mode.
  * PSUM->SBUF copies and exp() run on the idle Activation engine; the a_dst
    expansion one-hot (sexp) ships as fp8 (exact for one-hot).
  * Layer-2 table compute (Phase C) is fused into the Phase-B tile loop.
  * dma_gather needs int16 row ids, so the merged [8*NTP]-row tables are
    addressed as class A (row < 32768) and class B (row - 32768).
"""
import numpy as np
import ml_dtypes

import concourse.bass as bass
import concourse.mybir as mybir
import concourse.tile as tile
from concourse import library_config
from concourse.library_overlay import lower_extended_insts
from concourse.bass_utils import run_bass_kernel_spmd

# ---- problem constants (hardcoded per task contract) ----
N = 50000
E = 800000
IN_DIM = 256
HID = 64
HEADS = 4
H2 = 32
OUT_DIM = 16
G = 64
CORES = 8
GPC = G // CORES
NEG = 0.2
P = 128

ROW1 = 384   # L1 table row stride (bf16): [a_src(4)|a_dst(4)|h_int(256)|pad]
C1 = 264     # used columns of a L1 row
ROW2 = 128   # L2 table row stride (bf16): [a_src2(1)|a_dst2(1)|h2(32)|pad]
C2 = 34
NAR = 32768  # class-A rows (int16 index limit)

bf16 = mybir.dt.bfloat16
fp32 = mybir.dt.float32
fp8 = mybir.dt.float8e4
i16 = mybir.dt.int16
AF = mybir.ActivationFunctionType
OP = mybir.AluOpType


def _split_waits(nc, limit=1):
    """This walrus build rejects >1 sync-wait per instruction; hoist extras
    onto preceding same-engine NOPs (engine streams are in-order)."""
    n = 0
    for fn in nc.m.functions:
        for blk in fn.blocks:
            new_insts = []
            for inst in blk.instructions:
                si = getattr(inst, "sync_info", None)
                waits = list(si.on_wait) if si is not None else []
                if len(waits) > limit:
                    hoist, keep = waits[:-limit], waits[-limit:]
                    for j, w in enumerate(hoist):
                        nop = mybir.InstNoOp(
                            name=f"{inst.name}_wsplit{j}", ins=[], outs=[],
                            text_hint="waitsplit")
                        nop.engine = inst.engine
                        nop.sync_info = mybir.SyncInfo(on_wait=[w], on_update=[])
                        new_insts.append(nop)
                        n += 1
                    si.on_wait = keep
                new_insts.append(inst)
            blk.instructions[:] = new_insts
    return n


def build_nc(NT, KA, KB, emit_b1, emit_b2, reps=1, stop_after="Z"):
    """Build the SPMD kernel (same program for all 8 cores)."""
    NTP = NT * P
    K = KA + KB
    NIA, NIB = KA * P, KB * P
    ROWS = CORES * NTP
    NBR = ROWS - NAR
    assert NBR > 0

    nc = bass.Bass()

    # ---- I/O ----
    xtp_d = nc.dram_tensor("xtp", [NT, P, 2, P], bf16, kind="ExternalInput")
    w1e_d = nc.dram_tensor("w1e", [IN_DIM, C1], bf16, kind="ExternalInput")
    w2e_d = nc.dram_tensor("w2e", [IN_DIM, C2], bf16, kind="ExternalInput")
    idxa_d = nc.dram_tensor("idxa", [NT, P, max(NIA // 16, 1)], i16, kind="ExternalInput")
    idxb_d = nc.dram_tensor("idxb", [NT, P, max(NIB // 16, 1)], i16, kind="ExternalInput")
    loc_d = nc.dram_tensor("loc", [NT, P, K], bf16, kind="ExternalInput")
    sexp_d = nc.dram_tensor("sexp", [NT, P, K, P], fp8, kind="ExternalInput")
    pool_d = nc.dram_tensor("poolm", [NT, P, GPC], bf16, kind="ExternalInput")
    b1_d = nc.dram_tensor("b1r", [P, HEADS * HID], bf16, kind="ExternalInput")
    b2_d = nc.dram_tensor("b2r", [P, H2], bf16, kind="ExternalInput")
    iota_d = nc.dram_tensor("iotar", [P, P], bf16, kind="ExternalInput")
    ident_d = nc.dram_tensor("identr", [P, P], bf16, kind="ExternalInput")
    out_d = nc.dram_tensor("pooled", [GPC, H2], fp32, kind="ExternalOutput")

    # ---- internal DRAM (collectives) ----
    ag1_in = nc.dram_tensor("ag1_in", [NTP, ROW1], bf16)
    tab1 = nc.dram_tensor("tab1", [ROWS, ROW1], bf16, addr_space="Shared")
    ag2_in = nc.dram_tensor("ag2_in", [NTP, ROW2], bf16)
    tab2 = nc.dram_tensor("tab2", [ROWS, ROW2], bf16, addr_space="Shared")

    rg = [list(range(CORES))]

    with tile.TileContext(nc) as tc:
        with (
            tc.tile_pool(name="const", bufs=1) as cst,
            tc.tile_pool(name="io", bufs=3) as io,
            tc.tile_pool(name="wk", bufs=2) as wk,
            tc.tile_pool(name="gth", bufs=3) as gth,
            tc.tile_pool(name="big", bufs=1) as big,
            tc.tile_pool(name="ps", bufs=2, space="PSUM") as ps,
            tc.tile_pool(name="psg", bufs=1, space="PSUM") as psg,
        ):
            nc.gpsimd.load_library(library_config.mlp)
            nia_reg = nc.gpsimd.to_reg(NIA) if NIA else None
            nib_reg = nc.gpsimd.to_reg(NIB) if NIB else None

            iota_b = cst.tile([P, P], bf16)
            nc.sync.dma_start(out=iota_b[:], in_=iota_d[:])
            ident = cst.tile([P, P], bf16)
            nc.sync.dma_start(out=ident[:], in_=ident_d[:])

            w1e_sb = cst.tile([P, 2, C1], bf16)
            nc.sync.dma_start(out=w1e_sb[:], in_=w1e_d[:].rearrange("(a p) c -> p a c", p=P))
            w2e_sb = cst.tile([P, 2, C2], bf16)
            nc.sync.dma_start(out=w2e_sb[:], in_=w2e_d[:].rearrange("(a p) c -> p a c", p=P))
            if emit_b1:
                b1_sb = cst.tile([P, HEADS * HID], bf16)
                nc.sync.dma_start(out=b1_sb[:], in_=b1_d[:])
            if emit_b2:
                b2_sb = cst.tile([P, H2], bf16)
                nc.sync.dma_start(out=b2_sb[:], in_=b2_d[:])

            # local table slices stay SBUF-resident for adc / self-loop terms
            t1loc = big.tile([P, NT, C1], bf16)
            t2loc = big.tile([P, NT, C2], bf16)

            for rep in range(reps):
                # ================= Phase A: table1 slice =================
                for t in range(NT):
                    xt_t = io.tile([P, 2, P], bf16, tag="xt")
                    nc.sync.dma_start(out=xt_t[:], in_=xtp_d[t])
                    accA = ps.tile([P, C1], fp32, tag="acc")
                    for kk in range(2):
                        nc.tensor.matmul(
                            accA[:], xt_t[:, kk, :], w1e_sb[:, kk, :],
                            start=(kk == 0), stop=(kk == 1))
                    nc.scalar.copy(out=t1loc[:, t, :], in_=accA[:])
                    nc.sync.dma_start(
                        out=ag1_in[t * P:(t + 1) * P, 0:C1], in_=t1loc[:, t, :])
                if stop_after == "A0":
                    continue
                nc.gpsimd.collective_compute(
                    "AllGather", OP.bypass, ins=[ag1_in[:]], outs=[tab1[:]],
                    replica_groups=rg)
                if stop_after == "A":
                    continue

                # ====== Phase B: L1 message passing + fused table2 ======
                for t in range(NT):
                    g_all = gth.tile([P, K, ROW1], bf16, tag="g1")
                    if NIA:
                        idx_a = io.tile([P, NIA // 16], i16, tag="ixa")
                        nc.sync.dma_start(out=idx_a[:], in_=idxa_d[t])
                        nc.gpsimd.dma_gather(
                            out_ap=g_all[:, 0:KA, :], in_ap=tab1[0:NAR, :],
                            idxs_ap=idx_a[:], num_idxs=NIA, num_idxs_reg=nia_reg,
                            elem_size=ROW1, single_packet=False)
                    if NIB:
                        idx_b = io.tile([P, NIB // 16], i16, tag="ixb")
                        nc.sync.dma_start(out=idx_b[:], in_=idxb_d[t])
                        nc.gpsimd.dma_gather(
                            out_ap=g_all[:, KA:K, :], in_ap=tab1[NAR:ROWS, :],
                            idxs_ap=idx_b[:], num_idxs=NIB, num_idxs_reg=nib_reg,
                            elem_size=ROW1, single_packet=False)

                    loc_t = io.tile([P, K], bf16, tag="loc")
                    nc.sync.dma_start(out=loc_t[:], in_=loc_d[t])
                    s_all = wk.tile([P, K, P], bf16, tag="sel")
                    nc.vector.tensor_tensor(
                        out=s_all[:],
                        in0=loc_t[:].unsqueeze(2).to_broadcast([P, K, P]),
                        in1=iota_b[:].unsqueeze(1).to_broadcast([P, K, P]),
                        op=OP.is_equal)

                    sexp_t = wk.tile([P, K, P], fp8, tag="sx")
                    nc.sync.dma_start(out=sexp_t[:], in_=sexp_d[t])
                    adx = ps.tile([P, HEADS * K], fp32, tag="adx")
                    for j in range(K):
                        nc.tensor.matmul(
                            adx[:, 4 * j:4 * (j + 1)], sexp_t[:, j, :],
                            t1loc[:, t, 4:8], start=True, stop=True)
                    adxb = wk.tile([P, K, HEADS], bf16, tag="adxb")
                    nc.scalar.copy(
                        out=adxb[:], in_=adx[:].rearrange("p (k h) -> p k h", h=HEADS))

                    logit = wk.tile([P, K, HEADS], bf16, tag="lg1")
                    nc.vector.tensor_tensor(
                        out=logit[:], in0=g_all[:, :, 0:4], in1=adxb[:], op=OP.add)
                    lrl = wk.tile([P, K, HEADS], bf16, tag="lr1")
                    nc.vector.scalar_tensor_tensor(
                        out=lrl[:], in0=logit[:], scalar=NEG, in1=logit[:],
                        op0=OP.mult, op1=OP.max)

                    v_all = wk.tile([P, K, 260], bf16, tag="v1")
                    nc.scalar.activation(out=v_all[:, :, 0:4], in_=lrl[:], func=AF.Exp)
                    nc.vector.tensor_tensor(
                        out=v_all[:, :, 4:260].rearrange("p k (c h) -> p k c h", h=HEADS),
                        in0=g_all[:, :, 8:C1].rearrange("p k (c h) -> p k c h", h=HEADS),
                        in1=v_all[:, :, 0:4].unsqueeze(2).to_broadcast(
                            [P, K, HID, HEADS]),
                        op=OP.mult)

                    acc = ps.tile([P, 260], fp32, tag="acc")
                    for j in range(K):
                        nc.tensor.matmul(
                            acc[:], s_all[:, j, :], v_all[:, j, :],
                            start=(j == 0), stop=(j == K - 1))

                    # analytic self-loop term from the local slice
                    slg = wk.tile([P, HEADS], bf16, tag="slg")
                    nc.vector.tensor_tensor(
                        out=slg[:], in0=t1loc[:, t, 0:4], in1=t1loc[:, t, 4:8],
                        op=OP.add)
                    slr = wk.tile([P, HEADS], bf16, tag="slr")
                    nc.vector.scalar_tensor_tensor(
                        out=slr[:], in0=slg[:], scalar=NEG, in1=slg[:],
                        op0=OP.mult, op1=OP.max)
                    wsf = wk.tile([P, HEADS], bf16, tag="wsf")
                    nc.scalar.activation(out=wsf[:], in_=slr[:], func=AF.Exp)

                    dnm = wk.tile([P, HEADS], fp32, tag="dn1")
                    nc.vector.tensor_tensor(
                        out=dnm[:], in0=acc[:, 0:4], in1=wsf[:], op=OP.add)
                    rec = wk.tile([P, HEADS], fp32, tag="rc1")
                    nc.vector.reciprocal(rec[:], dnm[:])

                    sn1 = wk.tile([P, HID, HEADS], bf16, tag="sn1")
                    nc.vector.tensor_tensor(
                        out=sn1[:],
                        in0=t1loc[:, t, 8:C1].rearrange("p (c h) -> p c h", h=HEADS),
                        in1=wsf[:].unsqueeze(1).to_broadcast([P, HID, HEADS]),
                        op=OP.mult)
                    h1n = wk.tile([P, IN_DIM], fp32, tag="h1n")
                    nc.vector.tensor_tensor(
                        out=h1n[:].rearrange("p (c h) -> p c h", h=HEADS),
                        in0=acc[:, 4:260].rearrange("p (c h) -> p c h", h=HEADS),
                        in1=sn1[:], op=OP.add)
                    h1r = wk.tile([P, IN_DIM], bf16, tag="h1r")
                    nc.vector.tensor_tensor(
                        out=h1r[:].rearrange("p (c h) -> p c h", h=HEADS),
                        in0=h1n[:].rearrange("p (c h) -> p c h", h=HEADS),
                        in1=rec[:].unsqueeze(1).to_broadcast([P, HID, HEADS]),
                        op=OP.mult)
                    if emit_b1:
                        nc.vector.tensor_tensor(
                            out=h1r[:], in0=h1r[:], in1=b1_sb[:], op=OP.add)
                    nc.vector.tensor_scalar(
                        out=h1r[:], in0=h1r[:], scalar1=0.0, scalar2=None, op0=OP.max)

                    # fused Phase C: table2 row block for this tile
                    h1T = wk.tile([P, 2, P], bf16, tag="h1T")
                    for kk in range(2):
                        tr = ps.tile([P, P], bf16, tag="tr")
                        nc.tensor.transpose(
                            out=tr[:], in_=h1r[:, kk * P:(kk + 1) * P],
                            identity=ident[:])
                        nc.scalar.copy(out=h1T[:, kk, :], in_=tr[:])
                    acc2 = ps.tile([P, C2], fp32, tag="acc")
                    for kk in range(2):
                        nc.tensor.matmul(
                            acc2[:], h1T[:, kk, :], w2e_sb[:, kk, :],
                            start=(kk == 0), stop=(kk == 1))
                    nc.scalar.copy(out=t2loc[:, t, :], in_=acc2[:])
                    nc.sync.dma_start(
                        out=ag2_in[t * P:(t + 1) * P, 0:C2], in_=t2loc[:, t, :])

                if stop_after == "B":
                    continue
                nc.gpsimd.collective_compute(
                    "AllGather", OP.bypass, ins=[ag2_in[:]], outs=[tab2[:]],
                    replica_groups=rg)
                if stop_after == "C":
                    continue

                # ======== Phase D: L2 message passing + pooling ========
                gacc = psg.tile([GPC, H2], fp32, tag="gacc")
                for t in range(NT):
                    g2 = gth.tile([P, K, ROW2], bf16, tag="g2")
                    if NIA:
                        idx_a = io.tile([P, NIA // 16], i16, tag="ixa")
                        nc.sync.dma_start(out=idx_a[:], in_=idxa_d[t])
                        nc.gpsimd.dma_gather(
                            out_ap=g2[:, 0:KA, :], in_ap=tab2[0:NAR, :],
                            idxs_ap=idx_a[:], num_idxs=NIA, num_idxs_reg=nia_reg,
                            elem_size=ROW2, single_packet=False)
                    if NIB:
                        idx_b = io.tile([P, NIB // 16], i16, tag="ixb")
                        nc.sync.dma_start(out=idx_b[:], in_=idxb_d[t])
                        nc.gpsimd.dma_gather(
                            out_ap=g2[:, KA:K, :], in_ap=tab2[NAR:ROWS, :],
                            idxs_ap=idx_b[:], num_idxs=NIB, num_idxs_reg=nib_reg,
                            elem_size=ROW2, single_packet=False)

                    loc_t = io.tile([P, K], bf16, tag="loc")
                    nc.sync.dma_start(out=loc_t[:], in_=loc_d[t])
                    s_all = wk.tile([P, K, P], bf16, tag="sel")
                    nc.vector.tensor_tensor(
                        out=s_all[:],
                        in0=loc_t[:].unsqueeze(2).to_broadcast([P, K, P]),
                        in1=iota_b[:].unsqueeze(1).to_broadcast([P, K, P]),
                        op=OP.is_equal)

                    sexp_t = wk.tile([P, K, P], fp8, tag="sx")
                    nc.sync.dma_start(out=sexp_t[:], in_=sexp_d[t])
                    adx2 = ps.tile([P, K], fp32, tag="adx")
                    for j in range(K):
                        nc.tensor.matmul(
                            adx2[:, j:j + 1], sexp_t[:, j, :], t2loc[:, t, 1:2],
                            start=True, stop=True)
                    adx2b = wk.tile([P, K, 1], bf16, tag="adx2b")
                    nc.scalar.copy(out=adx2b[:], in_=adx2[:].unsqueeze(2))

                    lg2 = wk.tile([P, K, 1], bf16, tag="lg2")
                    nc.vector.tensor_tensor(
                        out=lg2[:], in0=g2[:, :, 0:1], in1=adx2b[:], op=OP.add)
                    lr2 = wk.tile([P, K, 1], bf16, tag="lr2")
                    nc.vector.scalar_tensor_tensor(
                        out=lr2[:], in0=lg2[:], scalar=NEG, in1=lg2[:],
                        op0=OP.mult, op1=OP.max)

                    v2 = wk.tile([P, K, 33], bf16, tag="v2")
                    nc.scalar.activation(out=v2[:, :, 0:1], in_=lr2[:], func=AF.Exp)
                    nc.vector.tensor_tensor(
                        out=v2[:, :, 1:33], in0=g2[:, :, 2:C2],
                        in1=v2[:, :, 0:1].to_broadcast([P, K, H2]), op=OP.mult)

                    accd = ps.tile([P, 33], fp32, tag="acc")
                    for j in range(K):
                        nc.tensor.matmul(
                            accd[:], s_all[:, j, :], v2[:, j, :],
                            start=(j == 0), stop=(j == K - 1))

                    sl2 = wk.tile([P, 1], bf16, tag="sl2")
                    nc.vector.tensor_tensor(
                        out=sl2[:], in0=t2loc[:, t, 0:1], in1=t2loc[:, t, 1:2],
                        op=OP.add)
                    sr2 = wk.tile([P, 1], bf16, tag="sr2")
                    nc.vector.scalar_tensor_tensor(
                        out=sr2[:], in0=sl2[:], scalar=NEG, in1=sl2[:],
                        op0=OP.mult, op1=OP.max)
                    ws2 = wk.tile([P, 1], bf16, tag="ws2")
                    nc.scalar.activation(out=ws2[:], in_=sr2[:], func=AF.Exp)

                    dn2 = wk.tile([P, 1], fp32, tag="dn2")
                    nc.vector.tensor_tensor(
                        out=dn2[:], in0=accd[:, 0:1], in1=ws2[:], op=OP.add)
                    rc2 = wk.tile([P, 1], fp32, tag="rc2")
                    nc.vector.reciprocal(rc2[:], dn2[:])

                    sn2 = wk.tile([P, H2], bf16, tag="sn2")
                    nc.vector.tensor_tensor(
                        out=sn2[:], in0=t2loc[:, t, 2:C2],
                        in1=ws2[:].to_broadcast([P, H2]), op=OP.mult)
                    h2n = wk.tile([P, H2], fp32, tag="h2n")
                    nc.vector.tensor_tensor(
                        out=h2n[:], in0=accd[:, 1:33], in1=sn2[:], op=OP.add)
                    h2p = wk.tile([P, H2], bf16, tag="h2p")
                    nc.vector.tensor_tensor(
                        out=h2p[:], in0=h2n[:],
                        in1=rc2[:].to_broadcast([P, H2]), op=OP.mult)
                    if emit_b2:
                        nc.vector.tensor_tensor(
                            out=h2p[:], in0=h2p[:], in1=b2_sb[:], op=OP.add)

                    pm = io.tile([P, GPC], bf16, tag="pm")
                    nc.sync.dma_start(out=pm[:], in_=pool_d[t])
                    nc.tensor.matmul(
                        gacc[:], pm[:], h2p[:], start=(t == 0), stop=(t == NT - 1))

                pooled = wk.tile([GPC, H2], fp32, tag="pooled")
                nc.scalar.copy(out=pooled[:], in_=gacc[:])
                nc.sync.dma_start(out=out_d[:], in_=pooled[:])

    lower_extended_insts(nc)
    _split_waits(nc)
    return nc


def _wrap16(idx_flat):
    """[NI] -> [P, NI//16] int16: position i -> partition i%16, col i//16,
    replicated across the 8 GPSIMD core groups."""
    ni = idx_flat.shape[0]
    w = np.zeros((P, max(ni // 16, 1)), np.int16)
    if ni:
        base = idx_flat.reshape(-1, 16).T.astype(np.int16)  # [16, ni/16]
        for grp in range(8):
            w[grp * 16:(grp + 1) * 16, :] = base
    return w


def _interleave_cols(M):
    """[*, HEADS*HID] head-major -> channel-major/head-minor interleave."""
    return M.reshape(M.shape[0], HEADS, HID).transpose(0, 2, 1).reshape(
        M.shape[0], HEADS * HID)


def preprocess(x, edge_index, batch, W1, att_src1, att_dst1, W2, att_src2, att_dst2):
    n0 = np.searchsorted(batch, np.arange(0, G + 1, GPC)).astype(np.int64)
    nodes_c = n0[1:] - n0[:-1]
    NT = int(np.ceil(nodes_c.max() / P))
    NTP = NT * P

    src = edge_index[0]
    dst = edge_index[1]

    owner = np.searchsorted(n0, np.arange(N), side="right") - 1
    ld_of = np.arange(N) - n0[owner]
    rowid = owner * NTP + ld_of          # merged-table row id
    isA = rowid < NAR

    percore = []
    KA = KB = 0
    for c in range(CORES):
        lo, hi = n0[c], n0[c + 1]
        sel = (dst >= lo) & (dst < hi)
        es, ed = src[sel], dst[sel]
        order = np.argsort(ed, kind="stable")
        es, ed = es[order], ed[order]
        ld = ed - lo
        tid = ld >> 7
        ea = isA[es]
        cntA = np.bincount(tid[ea], minlength=NT)
        cntB = np.bincount(tid[~ea], minlength=NT)
        KA = max(KA, int(np.ceil(cntA.max() / P)))
        KB = max(KB, int(np.ceil(cntB.max() / P)))
        percore.append((es, ld, tid, ea))

    K = KA + KB
    NIA, NIB, NI = KA * P, KB * P, K * P
    data = []
    for c in range(CORES):
        es, ld, tid, ea = percore[c]
        idxa = np.zeros((NT, P, max(NIA // 16, 1)), np.int16)
        idxb = np.zeros((NT, P, max(NIB // 16, 1)), np.int16)
        loc = np.full((NT, NI), 200.0, np.float32)
        for t in range(NT):
            m = tid == t
            iA = np.zeros(NIA, np.int64)
            iB = np.zeros(NIB, np.int64)
            locf = np.full(NI, 200.0, np.float32)
            mA = m & ea
            mB = m & ~ea
            nA, nB = int(mA.sum()), int(mB.sum())
            iA[:nA] = rowid[es[mA]]
            iB[:nB] = rowid[es[mB]] - NAR
            # slot order: A edges occupy subtiles [0,KA), B edges [KA,K)
            locf[:nA] = ld[mA] & 127
            locf[NIA:NIA + nB] = ld[mB] & 127
            idxa[t] = _wrap16(iA)
            idxb[t] = _wrap16(iB)
            loc[t] = locf
        lo, hi = n0[c], n0[c + 1]
        pm = np.zeros((NTP, GPC), np.float32)
        gl = batch[lo:hi] - c * GPC
        pm[np.arange(hi - lo), gl] = 1.0
        # pre-tiled x: xtp[t, feat_p, a, node] = x[lo + t*128 + node, a*128 + feat_p]
        xpad = np.zeros((NTP, IN_DIM), np.float32)
        xpad[:hi - lo] = x[lo:hi]
        xtp = xpad.reshape(NT, P, 2, P).transpose(0, 3, 2, 1)
        lockp = loc.reshape(NT, K, P)
        sexp = (np.arange(P)[None, :, None, None] == lockp[:, None, :, :])
        data.append(dict(
            idxa=idxa, idxb=idxb,
            loc=lockp.transpose(0, 2, 1).astype(ml_dtypes.bfloat16),
            sexp=sexp.astype(ml_dtypes.float8_e4m3),
            poolm=pm.reshape(NT, P, GPC).astype(ml_dtypes.bfloat16),
            xtp=np.ascontiguousarray(xtp).astype(ml_dtypes.bfloat16),
        ))
    cnts = np.stack([
        np.bincount(batch[n0[c]:n0[c + 1]] - c * GPC, minlength=GPC)
        for c in range(CORES)]).astype(np.float64)

    # fused weights: cols [a_src | a_dst | W(interleaved heads)]
    A1s = np.stack([W1[:, h * HID:(h + 1) * HID] @ att_src1[h] for h in range(HEADS)], 1)
    A1d = np.stack([W1[:, h * HID:(h + 1) * HID] @ att_dst1[h] for h in range(HEADS)], 1)
    W1i = _interleave_cols(W1)
    W1e = np.concatenate([A1s, A1d, W1i], axis=1).astype(ml_dtypes.bfloat16)
    A2s = (W2 @ att_src2[0])[:, None]
    A2d = (W2 @ att_dst2[0])[:, None]
    W2e = np.concatenate([A2s, A2d, W2], axis=1)
    # rows of W2e are indexed by h1 features: permute to interleaved order
    perm = (np.arange(HEADS * HID).reshape(HID, HEADS).T.reshape(-1))
    W2e = W2e[_inv_perm(perm)].astype(ml_dtypes.bfloat16)

    return dict(n0=n0, NT=NT, KA=KA, KB=KB, data=data, cnts=cnts, W1e=W1e, W2e=W2e)


def _inv_perm(p):
    ip = np.empty_like(p)
    ip[p] = np.arange(p.size)
    return ip


def make_in_maps(pp, b1, b2):
    b1i = _interleave_cols(b1[None, :])[0]
    b1r = np.tile(b1i[None, :], (P, 1)).astype(ml_dtypes.bfloat16)
    b2r = np.tile(b2[None, :], (P, 1)).astype(ml_dtypes.bfloat16)
    in_maps = []
    for c in range(CORES):
        d = pp["data"][c]
        in_maps.append({
            "xtp": d["xtp"], "w1e": pp["W1e"], "w2e": pp["W2e"],
            "idxa": d["idxa"], "idxb": d["idxb"], "sexp": d["sexp"],
            "loc": d["loc"], "poolm": d["poolm"], "b1r": b1r, "b2r": b2r,
            "iotar": np.tile(np.arange(P, dtype=np.float32), (P, 1)).astype(ml_dtypes.bfloat16),
            "identr": np.eye(P, dtype=np.float32).astype(ml_dtypes.bfloat16),
        })
    return in_maps


def epilogue(pooled_sums, pp, fc1_w, fc1_b, fc2_w, fc2_b):
    """pooled_sums: [CORES, GPC, H2] -> final [G, OUT_DIM] fp32."""
    cnts = pp["cnts"]  # [CORES, GPC]
    g = pooled_sums.astype(np.float64) / np.maximum(cnts, 1.0)[:, :, None]
    g = g.reshape(G, H2)
    g = np.maximum(g, 0.0)
    g = np.maximum(g @ fc1_w.astype(np.float64) + fc1_b, 0.0)
    g = np.maximum(g @ fc2_w.astype(np.float64) + fc2_b, 0.0)
    return g.astype(np.float32)


def kernel(**inputs):
    x = np.asarray(inputs["x"], np.float32)
    edge_index = np.asarray(inputs["edge_index"], np.int64)
    batch = np.asarray(inputs["batch"], np.int64)
    W1 = np.asarray(inputs["W1"], np.float32)
    att_src1 = np.asarray(inputs["att_src1"], np.float32)
    att_dst1 = np.asarray(inputs["att_dst1"], np.float32)
    b1 = np.asarray(inputs["b1"], np.float32)
    W2 = np.asarray(inputs["W2"], np.float32)
    att_src2 = np.asarray(inputs["att_src2"], np.float32)
    att_dst2 = np.asarray(inputs["att_dst2"], np.float32)
    b2 = np.asarray(inputs["b2"], np.float32)

    pp = preprocess(x, edge_index, batch, W1, att_src1, att_dst1, W2, att_src2, att_dst2)
    emit_b1 = bool(np.any(b1))
    emit_b2 = bool(np.any(b2))
    nc = build_nc(pp["NT"], pp["KA"], pp["KB"], emit_b1, emit_b2)
    in_maps = make_in_maps(pp, b1, b2)
    res = run_bass_kernel_spmd(nc, in_maps, list(range(CORES))).results
    pooled = np.stack([res[c]["pooled"] for c in range(CORES)])
    return epilogue(pooled, pp,
                    np.asarray(inputs["fc1_w"], np.float32), np.asarray(inputs["fc1_b"], np.float32),
                    np.asarray(inputs["fc2_w"], np.float32), np.asarray(inputs["fc2_b"], np.float32))
